# revision 1
# baseline (speedup 1.0000x reference)
"""Distributed flash-decoding attention kernel for 8 TRN2 NeuronCores.

Problem: B=1024 new tokens attend over a 32768-row KV cache plus the new
block (causal within the block). Sequence-parallel sharding: each core
handles 4096 cache rows + 128 new rows (4224 keys), computes partial
exp-scores @ V_aug (V with a ones column so the softmax normalizer comes
out of the same matmul), then a ReduceScatter combines partials and each
core emits its 128-query slice of the output.
"""

import os
import sys

import numpy as np

for _p in ("/opt/trn_rl_repo",):
    if os.path.isdir(_p) and _p not in sys.path:
        sys.path.insert(0, _p)

import ml_dtypes  # noqa: E402
import concourse.bacc as bacc  # noqa: E402
import concourse.mybir as mybir  # noqa: E402
import concourse.tile as tile  # noqa: E402
from concourse.bass_utils import run_bass_kernel_spmd  # noqa: E402

N_CORES = 8
B, S, DK, DV = 1024, 32768, 128, 128
S_SH = S // N_CORES  # 4096 cache rows per core
B_SH = B // N_CORES  # 128 new rows per core
NKEY = S_SH + B_SH  # 4224 keys per core
NT = NKEY // 128  # 33 key tiles
DVA = DV + 4  # 132: dv cols + l col (128) + 3 pad
QHW = 512  # q half width
F32 = mybir.dt.float32
F32R = mybir.dt.float32r
BF16 = mybir.dt.bfloat16
I32 = mybir.dt.int32

KT_CH = 11  # kt DMA chunks (3 key tiles each)
KT_W = (NT // KT_CH) * 128  # 384
VA_CH = 3  # vaug DMA chunks (11 key tiles each)
VA_T = NT // VA_CH  # 11


def _declare_io(nc):
    return dict(
        kt=nc.dram_tensor("kt", [128, NKEY], F32R, kind="ExternalInput"),
        qt=nc.dram_tensor("qt", [128, B], F32R, kind="ExternalInput"),
        vaug=nc.dram_tensor(
            "vaug", [NT, 128, DVA], BF16, kind="ExternalInput"
        ),
        thr=nc.dram_tensor("thr", [128, 1], F32, kind="ExternalInput"),
        out=nc.dram_tensor("out", [B_SH, DV], F32, kind="ExternalOutput"),
    )


def _emit_mask(nc, pmisc, th_d):
    """mask01[p, f] = 1.0 if query f >= (c*128 + p) else 0.0 (bf16)."""
    iota_i = pmisc.tile([128, B], I32, name="iota_i", tag="iota_i")
    nc.gpsimd.iota(iota_i[:], pattern=[[1, B]], base=0, channel_multiplier=0)
    iota_f = pmisc.tile([128, B], F32, name="iota_f", tag="iota_f")
    nc.vector.tensor_copy(iota_f[:], iota_i[:])
    thr_sb = pmisc.tile([128, 1], F32, name="thr", tag="thr")
    nc.sync.dma_start(thr_sb[:], th_d[:])
    mask01 = pmisc.tile([128, B], BF16, name="mask", tag="mask")
    nc.vector.tensor_scalar(
        out=mask01[:],
        in0=iota_f[:],
        scalar1=thr_sb[:],
        scalar2=None,
        op0=mybir.AluOpType.is_ge,
    )
    return mask01


def _emit_body(nc, pools, io, mask01, part, stage=4):
    """Loads + compute for one pass; writes the [B, DVA] partial to `part`.
    stage: 1=DMA, 2=+scores, 3=+exp, 4=full; 5=no-l, 6=l-only (timing)."""
    pkt, pqt, pva, pexp, ps_s = (
        pools["pkt"],
        pools["pqt"],
        pools["pva"],
        pools["pexp"],
        pools["ps_s"],
    )
    if stage < 1:
        return
    # Two HWDGE rings in parallel: kt + qt0 on the SP ring (nc.sync),
    # vaug + qt1 on the ACT ring (nc.scalar). First chunks of each stream
    # are small so the pipeline starts early.
    qt_sbs = []
    t0 = pqt.tile([128, QHW], F32R, name="qt0", tag="qt0")
    nc.sync.dma_start(t0[:], io["qt"][:, 0:QHW])
    qt_sbs.append(t0)
    va_sbs = []  # (first_tile_idx, n_tiles, tile)
    va_chunks = [(0, 3), (3, 15), (18, 15)]
    fi, n = va_chunks[0]
    va_t = pva.tile([128, n, DVA], BF16, name="va0", tag="va0")
    nc.scalar.dma_start(
        va_t[:], io["vaug"][fi : fi + n, :, :].rearrange("t p d -> p t d")
    )
    va_sbs.append((fi, n, va_t))
    t1 = pqt.tile([128, QHW], F32R, name="qt1", tag="qt1")
    nc.scalar.dma_start(t1[:], io["qt"][:, QHW : 2 * QHW])
    qt_sbs.append(t1)
    kt_sbs = []
    for i in range(KT_CH):
        t = pkt.tile([128, KT_W], F32R, name=f"kt{i}", tag=f"kt{i}")
        nc.sync.dma_start(t[:], io["kt"][:, i * KT_W : (i + 1) * KT_W])
        kt_sbs.append(t)
    for ci, (fi, n) in enumerate(va_chunks[1:], start=1):
        va_t = pva.tile([128, n, DVA], BF16, name=f"va{ci}", tag=f"va{ci}")
        nc.scalar.dma_start(
            va_t[:],
            io["vaug"][fi : fi + n, :, :].rearrange("t p d -> p t d"),
        )
        va_sbs.append((fi, n, va_t))

    def va_ap_for(t):
        for fi, n, tile_ in va_sbs:
            if fi <= t < fi + n:
                return tile_[:, t - fi, :]
        raise AssertionError(t)

    if stage < 2:
        return
    # pass A: scores + exp (resident, per q-half) + PV for q-subtiles 0..5
    # (6 PSUM banks) with per-half score tiles (2 banks, bufs=2).
    # pass B: PV for q-subtiles 6..7 re-reading the resident exp tiles.
    ps_oa = pools["ps_oa"]
    saved_e = []  # [t][qh] -> [128, 512] bf16
    oa = [
        ps_oa.tile([128, DVA], F32, name=f"oaA{qs}", tag=f"oa{qs}")
        for qs in range(6)
    ]

    def e_chunk(t, qs):
        return saved_e[t][qs // 4][:, (qs % 4) * 128 : (qs % 4 + 1) * 128]

    def pv(t, qs_list, accs, last):
        va_ap = va_ap_for(t)
        for i, qs in enumerate(qs_list):
            nc.tensor.matmul(
                accs[i][:],
                e_chunk(t, qs),
                va_ap,
                start=(t == 0),
                stop=last,
            )

    for t in range(NT):
        kt_ap = kt_sbs[t // 3][:, (t % 3) * 128 : (t % 3 + 1) * 128]
        es = []
        for qh in range(2):
            s_ps = ps_s.tile([128, QHW], F32, name="s", tag="s")
            nc.tensor.matmul(
                s_ps[:],
                kt_ap,
                qt_sbs[qh][:],
                start=True,
                stop=True,
            )
            if stage < 3:
                continue
            e_sb = pexp.tile(
                [128, QHW], BF16, name="e", tag="e", bufs=2 * NT
            )
            nc.scalar.activation(
                e_sb[:], s_ps[:], mybir.ActivationFunctionType.Exp
            )
            if t == NT - 1:
                e_m = pexp.tile([128, QHW], BF16, name="em", tag="em")
                nc.vector.tensor_tensor(
                    out=e_m[:],
                    in0=e_sb[:],
                    in1=mask01[:, qh * QHW : (qh + 1) * QHW],
                    op=mybir.AluOpType.mult,
                )
                e_sb = e_m
            es.append(e_sb)
        if stage < 3:
            continue
        saved_e.append(es)
        if stage < 4:
            continue
        if t >= 2:
            pv(t - 2, range(6), oa, last=False)
    if stage < 4:
        return
    pv(NT - 2, range(6), oa, last=False)
    pv(NT - 1, range(6), oa, last=True)
    for qs in range(6):
        oa_sb = pexp.tile([128, DVA], F32, name="oasb", tag="oasb")
        nc.vector.tensor_copy(oa_sb[:], oa[qs][:])
        nc.sync.dma_start(part[qs * 128 : (qs + 1) * 128, :], oa_sb[:])
    oa2 = [
        ps_oa.tile([128, DVA], F32, name=f"oaB{qs}", tag=f"oa{qs}")
        for qs in range(2)
    ]
    for t in range(NT):
        pv(t, (6, 7), oa2, last=(t == NT - 1))
    for qs in range(2):
        oa_sb = pexp.tile([128, DVA], F32, name="oasb2", tag="oasb")
        nc.vector.tensor_copy(oa_sb[:], oa2[qs][:])
        nc.sync.dma_start(part[(qs + 6) * 128 : (qs + 7) * 128, :], oa_sb[:])


def _emit_combine(nc, pep, part, red, out_d):
    nc.gpsimd.collective_compute(
        "ReduceScatter",
        mybir.AluOpType.add,
        replica_groups=[list(range(N_CORES))],
        ins=[part.opt()],
        outs=[red.opt()],
    )
    red_sb = pep.tile([B_SH, DVA], F32, name="red_sb", tag="red_sb")
    nc.sync.dma_start(red_sb[:], red[:])
    linv = pep.tile([B_SH, 1], F32, name="linv", tag="linv")
    nc.vector.reciprocal(linv[:], red_sb[:, DV : DV + 1])
    out_sb = pep.tile([B_SH, DV], F32, name="out_sb", tag="out_sb")
    nc.vector.tensor_scalar_mul(out_sb[:], red_sb[:, :DV], linv[:])
    nc.sync.dma_start(out_d[:], out_sb[:])


def build_nc(loop_iters: int | None = None, stage: int = 4):
    """loop_iters=None: real kernel (compute + ReduceScatter + epilogue).
    loop_iters=N: timing variant — compute body inside tc.For_i(0, N, 1),
    no collective (collectives can't sit inside control flow)."""
    nc = bacc.Bacc(
        "TRN2", target_bir_lowering=False, debug=False, num_devices=N_CORES
    )
    io = _declare_io(nc)
    with tile.TileContext(nc) as tc:
        with (
            tc.tile_pool(name="pkt", bufs=2) as pkt,
            tc.tile_pool(name="pqt", bufs=2) as pqt,
            tc.tile_pool(name="pva", bufs=2) as pva,
            tc.tile_pool(name="pexp", bufs=4) as pexp,
            tc.tile_pool(name="pmisc", bufs=1) as pmisc,
            tc.tile_pool(name="pep", bufs=2) as pep,
            tc.tile_pool(name="ps_s", bufs=2, space="PSUM") as ps_s,
            tc.tile_pool(name="ps_oa", bufs=1, space="PSUM") as ps_oa,
            tc.tile_pool(name="pdram", bufs=2, space="DRAM") as pdram,
        ):
            pools = dict(
                pkt=pkt, pqt=pqt, pva=pva, pexp=pexp, ps_s=ps_s, ps_oa=ps_oa
            )
            mask01 = _emit_mask(nc, pmisc, io["thr"])
            if loop_iters is None:
                part = pdram.tile([B, DVA], F32, name="part", tag="part")
                red = pdram.tile([B_SH, DVA], F32, name="red", tag="red")
                _emit_body(nc, pools, io, mask01, part)
                _emit_combine(nc, pep, part, red, io["out"])
            elif loop_iters == 0:
                # compute-only, single pass, no collective (for TimelineSim)
                part = pdram.tile([B, DVA], F32, name="part", tag="part")
                _emit_body(nc, pools, io, mask01, part)
                out_sb = pep.tile([B_SH, DV], F32, name="out_sb0", tag="out_sb")
                nc.vector.memset(out_sb[:], 0.0)
                nc.sync.dma_start(io["out"][:], out_sb[:])
            else:
                part = pdram.tile([B, DVA], F32, name="part", tag="part")
                with tc.For_i(0, loop_iters, 1):
                    _emit_body(nc, pools, io, mask01, part, stage=stage)
                # dummy output so the NEFF has a valid ExternalOutput write
                out_sb = pep.tile([B_SH, DV], F32, name="out_sb", tag="out_sb")
                nc.vector.memset(out_sb[:], 0.0)
                nc.sync.dma_start(io["out"][:], out_sb[:])
    nc.compile()
    return nc


_CACHE: dict = {}


def _get_nc():
    if "nc" not in _CACHE:
        _CACHE["nc"] = build_nc()
    return _CACHE["nc"]


def make_in_maps(q, k, v, K_cache, V_cache):
    q = np.asarray(q, np.float32)
    k = np.asarray(k, np.float32)
    v = np.asarray(v, np.float32)
    K_cache = np.asarray(K_cache, np.float32)
    V_cache = np.asarray(V_cache, np.float32)

    scale = 1.0 / np.sqrt(np.float32(DK))
    qt = np.ascontiguousarray((q * scale).T)  # [128, 1024]

    in_maps = []
    for c in range(N_CORES):
        Ksh = np.concatenate(
            [K_cache[c * S_SH : (c + 1) * S_SH], k[c * B_SH : (c + 1) * B_SH]],
            axis=0,
        )  # [4224, 128]
        kt = np.ascontiguousarray(Ksh.T)  # [128, 4224]
        Vsh = np.concatenate(
            [V_cache[c * S_SH : (c + 1) * S_SH], v[c * B_SH : (c + 1) * B_SH]],
            axis=0,
        )
        va = np.zeros((NKEY, DVA), np.float32)
        va[:, :DV] = Vsh
        va[:, DV] = 1.0
        va = va.reshape(NT, 128, DVA).astype(ml_dtypes.bfloat16)
        thr = (c * B_SH + np.arange(128, dtype=np.float32)).reshape(128, 1)
        in_maps.append({"kt": kt, "qt": qt, "vaug": va, "thr": thr})
    return in_maps


def kernel(q, k, v, K_cache, V_cache):
    in_maps = make_in_maps(q, k, v, K_cache, V_cache)
    res = run_bass_kernel_spmd(
        _get_nc(), in_maps, core_ids=list(range(N_CORES))
    )
    out = np.concatenate(
        [res.results[c]["out"] for c in range(N_CORES)], axis=0
    )
    return np.ascontiguousarray(out, dtype=np.float32)



# revision 7
# speedup vs baseline: 3.2723x; 3.2723x over previous
"""Distributed flash-decoding attention kernel for 8 TRN2 NeuronCores.

B=1024 new tokens attend over a 32768-row KV cache plus the new block
(causal within the block). Sequence-parallel: each core handles 4224 keys
(4096 cache + 128 new), all 1024 queries.

Per key tile t (128 keys), single pass:
  scores s = kt_t.T @ qt          -> PSUM f32 [128k, 1024q]  (2 MMs of 512)
  e = exp(s)                      -> SBUF bf16 (ACT, batched (2+1)/3 tiles)
  pv += va_t.T @ e                -> PSUM f32 [128dv, 1024q] (2 MMs of 512)
  acc += e                        -> SBUF bf16 (DVE, softmax normalizer)
l = ones.T @ acc (PE partition reduce); partial [dv|l, q] blocks go to a
[1032, 128] DRAM tensor; ReduceScatter over q-blocks; epilogue transposes
the received [128dv, 128q] block and scales by 1/l.

PSUM: 6 banks score ring (3 tile slots x 2 banks) + 2 banks PV accum.
"""

import os
import sys

import numpy as np

for _p in ("/opt/trn_rl_repo",):
    if os.path.isdir(_p) and _p not in sys.path:
        sys.path.insert(0, _p)

import ml_dtypes  # noqa: E402
import concourse.bacc as bacc  # noqa: E402
import concourse.mybir as mybir  # noqa: E402
import concourse.tile as tile  # noqa: E402
from concourse.bass_utils import run_bass_kernel_spmd  # noqa: E402
from concourse.masks import make_identity  # noqa: E402

N_CORES = 8
B, S, DK, DV = 1024, 32768, 128, 128
S_SH = S // N_CORES  # 4096 cache rows per core
B_SH = B // N_CORES  # 128 new rows per core
NKEY = S_SH + B_SH  # 4224 keys per core
NT = NKEY // 128  # 33 key tiles
RROW = DV + 1  # 129 rows per q-block in the reduce tensor (dv + l)
F32 = mybir.dt.float32
BF16 = mybir.dt.bfloat16
I32 = mybir.dt.int32

KT_CHUNKS = [(0, 4), (4, 8), (12, 8), (20, 7), (27, 6)]  # (first_tile, n)
VA_CHUNKS = [(0, 6), (6, 9), (15, 9), (24, 9)]


def _declare_io(nc):
    return dict(
        kt=nc.dram_tensor("kt", [128, NKEY], BF16, kind="ExternalInput"),
        qt=nc.dram_tensor("qt", [128, B], BF16, kind="ExternalInput"),
        va=nc.dram_tensor("va", [128, NKEY], BF16, kind="ExternalInput"),
        thr=nc.dram_tensor("thr", [128, 1], F32, kind="ExternalInput"),
        out=nc.dram_tensor("out", [B_SH, DV], F32, kind="ExternalOutput"),
    )


def _emit_mask(nc, pmisc, th_d):
    """mask01[p, f] = 1.0 if query f >= (c*128 + p) else 0.0 (bf16)."""
    iota_i = pmisc.tile([128, B], I32, name="iota_i", tag="iota_i")
    nc.gpsimd.iota(iota_i[:], pattern=[[1, B]], base=0, channel_multiplier=0)
    iota_f = pmisc.tile([128, B], F32, name="iota_f", tag="iota_f")
    nc.vector.tensor_copy(iota_f[:], iota_i[:])
    thr_sb = pmisc.tile([128, 1], F32, name="thr", tag="thr")
    nc.sync.dma_start(thr_sb[:], th_d[:])
    mask01 = pmisc.tile([128, B], BF16, name="mask", tag="mask")
    nc.vector.tensor_scalar(
        out=mask01[:],
        in0=iota_f[:],
        scalar1=thr_sb[:],
        scalar2=None,
        op0=mybir.AluOpType.is_ge,
    )
    return mask01


def _emit_body(nc, pools, io, mask01, part, stage=6, extras=None):
    """One pass of the compute body; writes the [1032, 128] partial to
    `part`. stage: 1=DMA only, 2=+scores, 3=+exp, 4=+PV, 5=+lacc,
    6=full (l reduce + copies + part DMA)."""
    p_in, p_e, p_acc, p_ep, ps_s, ps_pv = (
        pools["p_in"],
        pools["p_e"],
        pools["p_acc"],
        pools["p_ep"],
        pools["ps_s"],
        pools["ps_pv"],
    )

    # ---- input DMAs (chunked, all on the SP ring) ----
    qt_sb = p_in.tile([128, B], BF16, name="qt_sb", tag="qt")
    nc.sync.dma_start(qt_sb[:, 0:512], io["qt"][:, 0:512])
    kt_sb = p_in.tile([128, NKEY], BF16, name="kt_sb", tag="kt")
    f0, n0 = KT_CHUNKS[0]
    nc.sync.dma_start(
        kt_sb[:, f0 * 128 : (f0 + n0) * 128],
        io["kt"][:, f0 * 128 : (f0 + n0) * 128],
    )
    nc.sync.dma_start(qt_sb[:, 512:1024], io["qt"][:, 512:1024])
    va_sb = p_in.tile([128, NKEY], BF16, name="va_sb", tag="va")
    f0, n0 = VA_CHUNKS[0]
    nc.scalar.dma_start(
        va_sb[:, f0 * 128 : (f0 + n0) * 128],
        io["va"][:, f0 * 128 : (f0 + n0) * 128],
    )
    for f, n in KT_CHUNKS[1:]:
        nc.sync.dma_start(
            kt_sb[:, f * 128 : (f + n) * 128],
            io["kt"][:, f * 128 : (f + n) * 128],
        )
    for f, n in VA_CHUNKS[1:]:
        nc.scalar.dma_start(
            va_sb[:, f * 128 : (f + n) * 128],
            io["va"][:, f * 128 : (f + n) * 128],
        )
    if stage < 2:
        return

    # ---- PSUM score ring: 3 slots x [128, 1024] f32 (2 banks each) ----
    s_big = ps_s.tile([128, 3 * B], F32, name="s_big", tag="sbig")
    # e ring: 6 slots x [128, 1024] bf16
    e_buf = p_e.tile([128, 6 * B], BF16, name="e_buf", tag="ebuf")
    pv_ps = ps_pv.tile([128, B], F32, name="pv_ps", tag="pv")
    accs = []
    if stage >= 5:
        acc0 = p_acc.tile([128, B], BF16, name="acc0", tag="acc")
        nc.vector.memset(acc0[:], 0.0)
        accs.append(acc0)

    def e_slice(t):
        return e_buf[:, (t % 6) * B : (t % 6 + 1) * B]

    def emit_scores(t):
        j = t % 3
        kt_ap = kt_sb[:, t * 128 : (t + 1) * 128]
        for h in range(2):
            nc.tensor.matmul(
                s_big[:, j * B + h * 512 : j * B + (h + 1) * 512],
                kt_ap,
                qt_sb[:, h * 512 : (h + 1) * 512],
                start=True,
                stop=True,
            )
        if stage < 3:
            return
        # batched exp: tiles (3g, 3g+1) in one ACT instr, (3g+2) alone
        if j == 1:
            lo = (t - 1) % 6
            nc.scalar.activation(
                e_buf[:, lo * B : (lo + 2) * B],
                s_big[:, 0 : 2 * B],
                mybir.ActivationFunctionType.Exp,
            )
        elif j == 2:
            nc.scalar.activation(
                e_slice(t),
                s_big[:, 2 * B : 3 * B],
                mybir.ActivationFunctionType.Exp,
            )

    def emit_pv(tr):
        e_ap = e_slice(tr)
        if tr == NT - 1:
            em = p_e.tile([128, B], BF16, name="em", tag="em")
            nc.vector.tensor_tensor(
                out=em[:], in0=e_ap, in1=mask01[:], op=mybir.AluOpType.mult
            )
            e_ap = em[:]
        for h in range(2):
            nc.tensor.matmul(
                pv_ps[:, h * 512 : (h + 1) * 512],
                va_sb[:, tr * 128 : (tr + 1) * 128],
                e_ap[:, h * 512 : (h + 1) * 512],
                start=(tr == 0),
                stop=(tr == NT - 1),
            )
        if stage >= 5:
            nxt = p_acc.tile([128, B], BF16, name="accn", tag="acc")
            nc.vector.tensor_tensor(
                out=nxt[:], in0=accs[-1][:], in1=e_ap, op=mybir.AluOpType.add
            )
            accs.append(nxt)

    # PV lags scores by one 3-tile group so PE never queues a PV matmul
    # (which waits on ACT) ahead of ready score matmuls (FIFO stream).
    n_groups = (NT + 2) // 3  # 11
    for g in range(n_groups + 1):
        for j in range(3):
            t = 3 * g + j
            if t < NT:
                emit_scores(t)
        if stage >= 4 and g >= 1:
            for j in range(3):
                tr = 3 * (g - 1) + j
                if tr < NT:
                    emit_pv(tr)
    if stage < 6:
        return

    # ---- l = partition-reduce(acc) via ones-stationary matmul ----
    ones_sb = p_ep.tile([128, 1], BF16, name="ones_sb", tag="ones")
    nc.vector.memset(ones_sb[:], 1.0)
    l_ps = ps_s.tile([1, B], F32, name="l_ps", tag="sbig")
    for h in range(2):
        nc.tensor.matmul(
            l_ps[0:1, h * 512 : (h + 1) * 512],
            ones_sb[:],
            accs[-1][:, h * 512 : (h + 1) * 512],
            start=True,
            stop=True,
        )
    l_sb = p_ep.tile([1, B], F32, name="l_sb", tag="lsb")
    nc.vector.tensor_copy(l_sb[0:1, 0:512], l_ps[0:1, 0:512])
    nc.scalar.copy(l_sb[0:1, 512:1024], l_ps[0:1, 512:1024])
    if extras is not None:
        extras["acc"] = accs[-1]
        extras["l_sb"] = l_sb

    # ---- evacuate PV accum, DMA partial blocks ----
    pv_sb = p_ep.tile([128, B], F32, name="pv_sb", tag="pvsb")
    nc.vector.tensor_copy(pv_sb[:, 0:512], pv_ps[:, 0:512])
    nc.scalar.copy(pv_sb[:, 512:1024], pv_ps[:, 512:1024])

    part3 = part.rearrange("(j r) c -> j r c", r=RROW)
    nc.sync.dma_start(
        part3[:, 0:DV, :].rearrange("j r c -> r j c"),
        pv_sb[:].rearrange("p (j c) -> p j c", j=8),
    )
    nc.sync.dma_start(
        part3[:, DV : DV + 1, :].rearrange("j o c -> o j c"),
        l_sb[0:1, :].rearrange("o (j c) -> o j c", j=8),
    )


def _emit_epilogue(nc, pools, io, red):
    p_ep, ps_s, ps_pv = pools["p_ep"], pools["ps_s"], pools["ps_pv"]
    ident = pools["ident"]
    red_dv = p_ep.tile([128, DV], F32, name="red_dv", tag="red_dv")
    nc.sync.dma_start(red_dv[:], red[0:DV, :])
    red_l = p_ep.tile([1, B_SH], F32, name="red_l", tag="red_l")
    nc.sync.dma_start(red_l[:], red[DV : DV + 1, :])
    linv = p_ep.tile([1, B_SH], F32, name="linv", tag="linv")
    nc.vector.reciprocal(linv[:], red_l[:])
    one1 = p_ep.tile([1, 1], F32, name="one1", tag="one1")
    nc.vector.memset(one1[:], 1.0)

    t_ps = ps_s.tile([128, B_SH], F32, name="t_ps", tag="sbig")
    nc.tensor.transpose(t_ps[:], red_dv[:], ident[:])
    lc_ps = ps_pv.tile([128, 1], F32, name="lc_ps", tag="pv")
    nc.tensor.matmul(lc_ps[:], linv[:], one1[:], start=True, stop=True)
    lc_sb = p_ep.tile([128, 1], F32, name="lc_sb", tag="lc_sb")
    nc.vector.tensor_copy(lc_sb[:], lc_ps[:])
    out_sb = p_ep.tile([128, DV], F32, name="out_sb", tag="out_sb")
    nc.vector.tensor_scalar_mul(out_sb[:], t_ps[:], lc_sb[:])
    nc.sync.dma_start(io["out"][:], out_sb[:])


def build_nc(loop_iters: int | None = None, stage: int = 6):
    """loop_iters=None: real kernel (compute + ReduceScatter + epilogue).
    loop_iters=N: timing variant, compute body in tc.For_i (no
    collective -- collectives can't sit inside control flow)."""
    nc = bacc.Bacc(
        "TRN2", target_bir_lowering=False, debug=False, num_devices=N_CORES
    )
    io = _declare_io(nc)
    with tile.TileContext(nc) as tc:
        with (
            tc.tile_pool(name="p_in", bufs=1) as p_in,
            tc.tile_pool(name="p_e", bufs=1) as p_e,
            tc.tile_pool(name="p_acc", bufs=2) as p_acc,
            tc.tile_pool(name="pmisc", bufs=1) as pmisc,
            tc.tile_pool(name="p_ep", bufs=1) as p_ep,
            tc.tile_pool(name="ps_s", bufs=1, space="PSUM") as ps_s,
            tc.tile_pool(name="ps_pv", bufs=1, space="PSUM") as ps_pv,
            tc.tile_pool(name="pdram", bufs=1, space="DRAM") as pdram,
        ):
            pools = dict(
                p_in=p_in, p_e=p_e, p_acc=p_acc, p_ep=p_ep, ps_s=ps_s,
                ps_pv=ps_pv,
            )
            # ACT table prewarm: tiny exp before any real dependency
            warm = pmisc.tile([128, 1], F32, name="warm", tag="warm")
            nc.vector.memset(warm[:], 0.0)
            warm_o = pmisc.tile([128, 1], BF16, name="warm_o", tag="warm_o")
            nc.scalar.activation(
                warm_o[:], warm[:], mybir.ActivationFunctionType.Exp
            )
            ident = pmisc.tile([128, 128], F32, name="ident", tag="ident")
            make_identity(nc, ident[:])
            pools["ident"] = ident
            mask01 = _emit_mask(nc, pmisc, io["thr"])
            if loop_iters is None:
                part = pdram.tile([8 * RROW, B_SH], F32, name="part", tag="pa")
                red = pdram.tile([RROW, B_SH], F32, name="red", tag="re")
                _emit_body(nc, pools, io, mask01[:], part)
                nc.gpsimd.collective_compute(
                    "ReduceScatter",
                    mybir.AluOpType.add,
                    replica_groups=[list(range(N_CORES))],
                    ins=[part.opt()],
                    outs=[red.opt()],
                )
                _emit_epilogue(nc, pools, io, red)
            else:
                part = pdram.tile([8 * RROW, B_SH], F32, name="part", tag="pa")
                with tc.For_i(0, max(loop_iters, 1), 1):
                    _emit_body(nc, pools, io, mask01[:], part, stage=stage)
                out_sb = p_ep.tile([B_SH, DV], F32, name="out_sb0", tag="o0")
                nc.vector.memset(out_sb[:], 0.0)
                nc.sync.dma_start(io["out"][:], out_sb[:])
    nc.compile()
    return nc


_CACHE: dict = {}


def _get_nc():
    if "nc" not in _CACHE:
        _CACHE["nc"] = build_nc()
    return _CACHE["nc"]


def make_in_maps(q, k, v, K_cache, V_cache):
    q = np.asarray(q, np.float32)
    k = np.asarray(k, np.float32)
    v = np.asarray(v, np.float32)
    K_cache = np.asarray(K_cache, np.float32)
    V_cache = np.asarray(V_cache, np.float32)

    scale = 1.0 / np.sqrt(np.float32(DK))
    qt = np.ascontiguousarray((q * scale).T).astype(ml_dtypes.bfloat16)

    in_maps = []
    for c in range(N_CORES):
        Ksh = np.concatenate(
            [K_cache[c * S_SH : (c + 1) * S_SH], k[c * B_SH : (c + 1) * B_SH]],
            axis=0,
        )  # [4224, 128]
        kt = np.ascontiguousarray(Ksh.T).astype(ml_dtypes.bfloat16)
        Vsh = np.concatenate(
            [V_cache[c * S_SH : (c + 1) * S_SH], v[c * B_SH : (c + 1) * B_SH]],
            axis=0,
        )  # [4224, 128]
        # va[p, t*128 + d] = V[t*128 + p, d]  (PE stationary layout)
        va = np.ascontiguousarray(
            Vsh.reshape(NT, 128, DV).transpose(1, 0, 2).reshape(128, NKEY)
        ).astype(ml_dtypes.bfloat16)
        thr = (c * B_SH + np.arange(128, dtype=np.float32)).reshape(128, 1)
        in_maps.append({"kt": kt, "qt": qt, "va": va, "thr": thr})
    return in_maps


def kernel(q, k, v, K_cache, V_cache):
    in_maps = make_in_maps(q, k, v, K_cache, V_cache)
    res = run_bass_kernel_spmd(
        _get_nc(), in_maps, core_ids=list(range(N_CORES))
    )
    out = np.concatenate(
        [res.results[c]["out"] for c in range(N_CORES)], axis=0
    )
    return np.ascontiguousarray(out, dtype=np.float32)
